# revision 2
# baseline (speedup 1.0000x reference)
"""Decoder layer on 8 trn2 cores — v2.

Sharding: core c = 2*b + g. Each core owns batch b and the balanced q-chunk
pair {0,3} (g=0) or {1,2} (g=1) of the 4x512-token chunks, so every core has
identical causal work. Host permutes tokens to local order [A|B|rest1|rest2]
(A = lower own chunk, B = upper own chunk, rest = other core's chunks
ascending); visibility is then uniform:
  chunk A: tiles 0-3 triangular, tiles 8-11 gated by pb1 (data: 0 / -1e30)
  chunk B: tiles 0-3 full, 4-7 triangular, 8-11 full, 12-15 gated by pb2
K/V are computed for all 2048 tokens of the batch (recompute, zero
collectives). Everything runs transposed [D, tokens]; weights arrive as
pre-tiled bf16 panels so each weight DMA is one [128, 1024/4096] transfer.

Packing: scores row-pack head pairs (two K=64 matmuls in one slot), AV
col-packs them (two M=64), softmax denominators ride as packed M=1 matmuls.
V's bias is folded into bo on the host; bo + residual arrive as a
precomputed x_resid input; remaining biases are applied as per-partition
ACT-evac biases. LN rsqrt = exp(-.5*ln(var+eps)) so the whole LN+attention
era uses one ACT table set; the MLP switches once to the gelu set (chunk A's
gelu is deferred past the last exp to avoid table thrash).
"""

import numpy as np

D = 1024
H = 16
DH = 64
TKV = 2048
DFF = 4096
EPS = 1e-5
NEG = -1.0e30
KT = 8  # 1024 / 128

_CACHE = {}

# attention tile lists: (kt, kind) kind: 0=full, 1=triangle, 2=gate1, 3=gate2
VIS_A = [(0, 1), (1, 1), (2, 1), (3, 1), (8, 2), (9, 2), (10, 2), (11, 2)]
VIS_B = ([(k, 0) for k in range(4)] + [(k, 1) for k in range(4, 8)]
         + [(k, 0) for k in range(8, 12)] + [(k, 3) for k in range(12, 16)])


def _build():
    if "nc" in _CACHE:
        return _CACHE["nc"]
    import concourse.mybir as mybir
    import concourse.tile as tile
    from concourse import bacc
    import contextlib

    f32 = mybir.dt.float32
    f32r = mybir.dt.float32r
    bf16 = mybir.dt.bfloat16
    Act = mybir.ActivationFunctionType
    Alu = mybir.AluOpType

    nc = bacc.Bacc(None, target_bir_lowering=False)

    xT = nc.declare_dram_parameter("xT", [D, TKV], bf16, isOutput=False)
    xres = nc.declare_dram_parameter("xres", [D, 1024], bf16, isOutput=False)
    pb1 = nc.declare_dram_parameter("pb1", [128, 1], f32, isOutput=False)
    pb2 = nc.declare_dram_parameter("pb2", [128, 1], f32, isOutput=False)
    tri = nc.declare_dram_parameter("tri", [128, 128], bf16, isOutput=False)
    id128 = nc.declare_dram_parameter("id128", [128, 128], bf16, isOutput=False)
    onesc = nc.declare_dram_parameter("onesc", [128, 1], bf16, isOutput=False)
    onescr = nc.declare_dram_parameter("onescr", [128, 1], f32r, isOutput=False)
    onesr = nc.declare_dram_parameter("onesr", [33, 128], bf16, isOutput=False)
    wq_t = nc.declare_dram_parameter("wq_t", [KT, 128, D], bf16, isOutput=False)
    wk_t = nc.declare_dram_parameter("wk_t", [KT, 128, D], bf16, isOutput=False)
    wv_t = nc.declare_dram_parameter("wv_t", [2, 128, 4096], bf16, isOutput=False)
    wo_t = nc.declare_dram_parameter("wo_t", [KT, 128, D], bf16, isOutput=False)
    wfc_t = nc.declare_dram_parameter("wfc_t", [32, 128, D], bf16, isOutput=False)
    wpr_t = nc.declare_dram_parameter("wpr_t", [KT, 128, DFF], bf16, isOutput=False)
    bq_d = nc.declare_dram_parameter("bq_d", [128, KT], f32, isOutput=False)
    bk_d = nc.declare_dram_parameter("bk_d", [128, KT], f32, isOutput=False)
    bfc_d = nc.declare_dram_parameter("bfc_d", [128, 32], f32, isOutput=False)
    bpr_d = nc.declare_dram_parameter("bpr_d", [128, KT], f32, isOutput=False)
    out_T = nc.declare_dram_parameter("out_T", [D, 1024], f32, isOutput=True)

    with tile.TileContext(nc) as tc, contextlib.ExitStack() as root:
        cst = root.enter_context(tc.tile_pool(name="cst", bufs=1))
        # ---------------- constants
        # NOTE: ones_c/ones_r DMA'd FIRST — with them placed later in the
        # pool, the M=1 bf16 stats matmuls produced corrupted even columns
        # (HW quirk, empirically layout-dependent; see session notes).
        ones_c = cst.tile([128, 1], bf16, tag="onesc")
        nc.sync.dma_start(out=ones_c[:], in_=onesc[:])
        ones_r = cst.tile([33, 128], bf16, tag="onesr")
        nc.sync.dma_start(out=ones_r[:], in_=onesr[:])
        tri_t = cst.tile([128, 128], bf16, tag="tri")
        nc.sync.dma_start(out=tri_t[:], in_=tri[:])
        id_t = cst.tile([128, 128], bf16, tag="id")
        nc.sync.dma_start(out=id_t[:], in_=id128[:])
        ones_cr = cst.tile([128, 1], f32r, tag="onescr")
        nc.sync.dma_start(out=ones_cr[:], in_=onescr[:])
        pb1_t = cst.tile([128, 1], f32, tag="pb1")
        nc.sync.dma_start(out=pb1_t[:], in_=pb1[:])
        pb2_t = cst.tile([128, 1], f32, tag="pb2")
        nc.sync.dma_start(out=pb2_t[:], in_=pb2[:])
        bq_t = cst.tile([128, KT], f32, tag="bq")
        nc.sync.dma_start(out=bq_t[:], in_=bq_d[:])
        bk_t = cst.tile([128, KT], f32, tag="bk")
        nc.sync.dma_start(out=bk_t[:], in_=bk_d[:])
        bfc_t = cst.tile([128, 32], f32, tag="bfc")
        nc.sync.dma_start(out=bfc_t[:], in_=bfc_d[:])
        bpr_t = cst.tile([128, KT], f32, tag="bpr")
        nc.sync.dma_start(out=bpr_t[:], in_=bpr_d[:])
        eps_t = cst.tile([33, 1], f32, tag="eps")
        nc.any.memset(eps_t[:], EPS)

        def layernorm(s, src_tiles, ncols, out_tiles, f32_src):
            """src tiles [128, ncols] -> normalized out tiles (bf16)."""
            nch = ncols // 512
            pst = s.enter_context(tc.tile_pool(name="pst", bufs=1, space="PSUM"))
            pbc = s.enter_context(tc.tile_pool(name="pbc", bufs=2, space="PSUM"))
            stp = s.enter_context(tc.tile_pool(name="stp", bufs=2))
            bcp = s.enter_context(tc.tile_pool(name="bcp", bufs=1))
            sqp = s.enter_context(tc.tile_pool(name="sqp", bufs=1))
            lhs_x = ones_cr if f32_src else ones_c
            mu_bc = bcp.tile([128, ncols], bf16, tag="mubc")
            rs_bc = bcp.tile([128, ncols], bf16, tag="rsbc")
            for half in range((nch + 1) // 2):
                cs = [c for c in (2 * half, 2 * half + 1) if c < nch]
                psx = {c: pst.tile([1, 512], f32, tag=f"psx{c % 2}",
                                   name=f"psx{c}") for c in cs}
                psq = {c: pst.tile([1, 512], f32, tag=f"psq{c % 2}",
                                   name=f"psq{c}") for c in cs}
                for k in range(KT):
                    a = src_tiles[k][:, 1024 * half:1024 * half + 512 * len(cs)]
                    if f32_src:
                        a = a.bitcast(f32)
                    sq = sqp.tile([128, 512 * len(cs)], bf16, tag="sqh")
                    nc.vector.tensor_tensor(sq[:], a, a, Alu.mult)
                    for j, c in enumerate(cs):
                        nc.tensor.matmul(
                            psx[c][:], lhs_x[:],
                            src_tiles[k][:, c * 512:(c + 1) * 512],
                            start=(k == 0), stop=(k == KT - 1))
                        nc.tensor.matmul(
                            psq[c][:], ones_c[:],
                            sq[:, j * 512:(j + 1) * 512],
                            start=(k == 0), stop=(k == KT - 1))
                # stats math immediately per half (frees psum banks)
                for c in cs:
                    ps_x, ps_q = psx[c], psq[c]
                    mu_sb = stp.tile([1, 512], bf16, tag="mu", name=f"mu{c}")
                    nc.scalar.mul(mu_sb[:], ps_x[:], 1.0 / D)
                    t_sb = stp.tile([1, 512], f32, tag="t")
                    nc.vector.tensor_tensor(t_sb[:], ps_x[:], mu_sb[:],
                                            Alu.mult)
                    v_t = stp.tile([1, 512], f32, tag="v")
                    nc.vector.tensor_tensor(v_t[:], ps_q[:], t_sb[:],
                                            Alu.subtract)
                    lnv = stp.tile([1, 512], f32, tag="lnv")
                    nc.scalar.activation(lnv[:], v_t[:], Act.Ln,
                                         bias=eps_t[0:1, 0:1], scale=1.0 / D)
                    rs_sb = stp.tile([1, 512], bf16, tag="rs", name=f"rs{c}")
                    nc.scalar.activation(rs_sb[:], lnv[:], Act.Exp, bias=0.0,
                                         scale=-0.5)
                    pm = pbc.tile([128, 512], f32, tag="pm")
                    nc.tensor.matmul(pm[:], ones_r[0:1, :], mu_sb[:],
                                     start=True, stop=True)
                    nc.vector.tensor_copy(mu_bc[:, c * 512:(c + 1) * 512],
                                          pm[:])
                    pr_ = pbc.tile([128, 512], f32, tag="pm")
                    nc.tensor.matmul(pr_[:], ones_r[0:1, :], rs_sb[:],
                                     start=True, stop=True)
                    nc.vector.tensor_copy(rs_bc[:, c * 512:(c + 1) * 512],
                                          pr_[:])
            dtp = s.enter_context(tc.tile_pool(name="dtp", bufs=1))
            for k in range(KT):
                a = src_tiles[k][:]
                if f32_src:
                    a = src_tiles[k][:].bitcast(f32)
                d_t = dtp.tile([128, ncols], bf16, tag="d")
                nc.vector.tensor_tensor(d_t[:], a, mu_bc[:], Alu.subtract)
                nc.vector.tensor_tensor(out_tiles[k][:], d_t[:], rs_bc[:],
                                        Alu.mult)

        # long-lived pools, opened in LIFO-close order:
        # x1/h2 close at root exit, attn after WO, kv after attention
        es_x1 = contextlib.ExitStack()
        x1p = es_x1.enter_context(tc.tile_pool(name="x1", bufs=1))
        x1_sb = [x1p.tile([128, 1024], bf16, tag=f"x1{m}", name=f"x1{m}")
                 for m in range(KT)]
        es_h2 = contextlib.ExitStack()
        h2p = es_h2.enter_context(tc.tile_pool(name="h2", bufs=1))
        h2_sb = [h2p.tile([128, 1024], bf16, tag=f"h2{m}", name=f"h2{m}")
                 for m in range(KT)]
        es_att = contextlib.ExitStack()
        attnp = es_att.enter_context(tc.tile_pool(name="attn", bufs=1))
        at_A = [attnp.tile([128, 512], bf16, tag=f"aA{m}", name=f"aA{m}")
                for m in range(KT)]
        at_B = [attnp.tile([128, 512], bf16, tag=f"aB{m}", name=f"aB{m}")
                for m in range(KT)]

        # ================= Phase 1: LN1 + Q/K/V =================
        es_kv = contextlib.ExitStack()
        kvp = es_kv.enter_context(tc.tile_pool(name="kv", bufs=1))
        kT_sb = [kvp.tile([128, TKV], bf16, tag=f"kT{m}", name=f"kT{m}")
                 for m in range(KT)]
        qT_sb = [kvp.tile([128, 1024], bf16, tag=f"qT{m}", name=f"qT{m}")
                 for m in range(KT)]
        v_sb = [kvp.tile([128, D], bf16, tag=f"v{t}", name=f"vt{t}")
                for t in range(16)]

        with contextlib.ExitStack() as s1:
            xp = s1.enter_context(tc.tile_pool(name="xp", bufs=1))
            x_t = []
            for k in range(KT):
                t = xp.tile([128, TKV], bf16, tag=f"x{k}", name=f"x{k}")
                nc.sync.dma_start(out=t[:], in_=xT[k * 128:(k + 1) * 128, :])
                x_t.append(t)
            with contextlib.ExitStack() as s1a:
                # normalize in place: h == x tiles after this
                layernorm(s1a, x_t, TKV, x_t, f32_src=False)
            h_t = x_t

            wpp = s1.enter_context(tc.tile_pool(name="wpp", bufs=3))
            wvp = s1.enter_context(tc.tile_pool(name="wvp", bufs=1))
            pmm = s1.enter_context(tc.tile_pool(name="pmm", bufs=3, space="PSUM"))

            # Q (own 1024 tokens = local cols 0..1023)
            for mc in range(KT):
                wp = wpp.tile([128, D], bf16, tag="wpan")
                nc.sync.dma_start(out=wp[:], in_=wq_t[mc])
                for c in range(2):
                    ps = pmm.tile([128, 512], f32, tag="pmm")
                    for k in range(KT):
                        nc.tensor.matmul(ps[:], wp[:, k * 128:(k + 1) * 128],
                                         h_t[k][:, c * 512:(c + 1) * 512],
                                         start=(k == 0), stop=(k == KT - 1))
                    nc.scalar.activation(qT_sb[mc][:, c * 512:(c + 1) * 512],
                                         ps[:], Act.Identity,
                                         bias=bq_t[:, mc:mc + 1])
            # K (all 2048)
            for mc in range(KT):
                wp = wpp.tile([128, D], bf16, tag="wpan")
                nc.sync.dma_start(out=wp[:], in_=wk_t[mc])
                for c in range(4):
                    ps = pmm.tile([128, 512], f32, tag="pmm")
                    for k in range(KT):
                        nc.tensor.matmul(ps[:], wp[:, k * 128:(k + 1) * 128],
                                         h_t[k][:, c * 512:(c + 1) * 512],
                                         start=(k == 0), stop=(k == KT - 1))
                    nc.scalar.activation(kT_sb[mc][:, c * 512:(c + 1) * 512],
                                         ps[:], Act.Identity,
                                         bias=bk_t[:, mc:mc + 1])
            # V (all 2048 tokens, token-major [tok, D])
            wv_sb = []
            for c in range(2):
                wv_c = wvp.tile([128, 4096], bf16, tag=f"wvpan{c}",
                                name=f"wvpan{c}")
                nc.sync.dma_start(out=wv_c[:], in_=wv_t[c])
                wv_sb.append(wv_c)
            v_order = [0, 1, 2, 3, 8, 9, 10, 11, 4, 5, 6, 7, 12, 13, 14, 15]
            for tl in v_order:
                for c in range(2):
                    ps = pmm.tile([128, 512], f32, tag="pmm")
                    for k in range(KT):
                        nc.tensor.matmul(
                            ps[:], h_t[k][:, tl * 128:(tl + 1) * 128],
                            wv_sb[c][:, k * 512:(k + 1) * 512],
                            start=(k == 0), stop=(k == KT - 1))
                    nc.vector.tensor_copy(v_sb[tl][:, c * 512:(c + 1) * 512],
                                          ps[:])

        # ================= Phase 2: attention =================

        def attn_chunk(pools, vis, qc, at_out):
            # PSUM discipline: one accumulation chain per bank; the first
            # matmul of each chain covers the chain's full column range
            # (mask preloads ACCUMULATE after the scores matmul instead of
            # preceding it).
            pss, pav, pdnb, etp, rcp = pools
            for hp_ in range(KT):
                work = []
                for kt, kind in vis:
                    c0 = (kt % 4) * 128 if kind == 1 else 0
                    ps = pss.tile([128, 1024], f32, tag="pss")
                    for hh in range(2):
                        lo, hi = hh * 64, hh * 64 + 64
                        nc.tensor.matmul(
                            ps[:, 512 * hh + c0:512 * hh + 512],
                            kT_sb[hp_][lo:hi, kt * 128:(kt + 1) * 128],
                            qT_sb[hp_][lo:hi, qc * 512 + c0:qc * 512 + 512],
                            start=True, stop=(kind != 1))
                        if kind == 1:
                            nc.tensor.matmul(
                                ps[:, 512 * hh + c0:512 * hh + c0 + 128],
                                id_t[:], tri_t[:], start=False, stop=True)
                    et = etp.tile([128, 1024], bf16, tag="et")
                    bias = 0.0
                    if kind == 2:
                        bias = pb1_t[:, 0:1]
                    elif kind == 3:
                        bias = pb2_t[:, 0:1]
                    nc.scalar.activation(et[:, c0:1024], ps[:, c0:1024],
                                         Act.Exp, bias=bias, scale=0.125)
                    work.append((kt, c0, et))
                ps_av0 = pav.tile([128, 512], f32, tag="pav0")
                ps_av1 = pav.tile([128, 512], f32, tag="pav1")
                ps_d0 = pdnb.tile([128, 512], f32, tag="dnb", name="psd0")
                ps_d1 = pdnb.tile([128, 512], f32, tag="dnb", name="psd1")
                n = len(work)
                for i, (kt, c0, et) in enumerate(work):
                    st, sp = (i == 0), (i == n - 1)
                    nc.tensor.matmul(
                        ps_av0[0:64, c0:512],
                        v_sb[kt][:, hp_ * 128:hp_ * 128 + 64],
                        et[:, c0:512], start=st, stop=sp)
                    nc.tensor.matmul(
                        ps_av1[64:128, c0:512],
                        v_sb[kt][:, hp_ * 128 + 64:hp_ * 128 + 128],
                        et[:, 512 + c0:1024], start=st, stop=sp)
                    nc.tensor.matmul(ps_d0[0:1, c0:512], ones_c[:],
                                     et[:, c0:512], start=st, stop=sp)
                    nc.tensor.matmul(ps_d1[32:33, c0:512], ones_c[:],
                                     et[:, 512 + c0:1024], start=st, stop=sp)
                rec = rcp.tile([33, 512], bf16, tag="rec")
                with nc.allow_low_precision(reason="softmax denom bf16"):
                    nc.vector.reciprocal(rec[0:1, :], ps_d0[0:1, :])
                    nc.vector.reciprocal(rec[32:33, :], ps_d1[32:33, :])
                ps_b = pdnb.tile([128, 512], f32, tag="dnb", name="psb")
                nc.tensor.matmul(ps_b[0:64, :], ones_r[0:1, 0:64],
                                 rec[0:1, :], start=True, stop=True)
                nc.tensor.matmul(ps_b[64:128, :], ones_r[32:33, 0:64],
                                 rec[32:33, :], start=True, stop=True)
                bc_sb = rcp.tile([128, 512], f32, tag="bcsb")
                nc.vector.tensor_copy(bc_sb[:], ps_b[:])
                nc.vector.tensor_tensor(at_out[hp_][0:64, :], ps_av0[0:64, :],
                                        bc_sb[0:64, :], Alu.mult)
                nc.vector.tensor_tensor(at_out[hp_][64:128, :],
                                        ps_av1[64:128, :],
                                        bc_sb[64:128, :], Alu.mult)

        with contextlib.ExitStack() as s2:
            pss = s2.enter_context(tc.tile_pool(name="pss", bufs=2, space="PSUM"))
            pav = s2.enter_context(tc.tile_pool(name="pav", bufs=1, space="PSUM"))
            pdnb = s2.enter_context(tc.tile_pool(name="pdnb", bufs=2, space="PSUM"))
            etp = s2.enter_context(tc.tile_pool(name="etp", bufs=4))
            rcp = s2.enter_context(tc.tile_pool(name="rcp", bufs=2))
            pools = (pss, pav, pdnb, etp, rcp)
            attn_chunk(pools, VIS_A, 0, at_A)
            attn_chunk(pools, VIS_B, 1, at_B)

        es_kv.close()  # free kT/qT/v

        # ================= Phase 3: WO + LN2 =================
        with contextlib.ExitStack() as s3:
            xrp = s3.enter_context(tc.tile_pool(name="xrp", bufs=1))
            wpp3 = s3.enter_context(tc.tile_pool(name="wpp3", bufs=3))
            pwo = s3.enter_context(tc.tile_pool(name="pwo", bufs=2, space="PSUM"))
            xr_t = []
            for m in range(KT):
                t = xrp.tile([128, 1024], bf16, tag=f"xr{m}", name=f"xr{m}")
                nc.sync.dma_start(out=t[:], in_=xres[m * 128:(m + 1) * 128, :])
                xr_t.append(t)
            for mc in range(KT):
                wp = wpp3.tile([128, D], bf16, tag="wpan3")
                nc.sync.dma_start(out=wp[:], in_=wo_t[mc])
                for c, at_ in enumerate((at_A, at_B)):
                    ps = pwo.tile([128, 512], f32, tag="pwo")
                    for k in range(KT):
                        nc.tensor.matmul(ps[:], wp[:, k * 128:(k + 1) * 128],
                                         at_[k][:], start=(k == 0),
                                         stop=(k == KT - 1))
                    nc.vector.tensor_tensor(
                        x1_sb[mc][:, c * 512:(c + 1) * 512], ps[:],
                        xr_t[mc][:, c * 512:(c + 1) * 512], Alu.add)

        es_att.close()  # free at_A/at_B

        with contextlib.ExitStack() as s3b:
            layernorm(s3b, x1_sb, 1024, h2_sb, f32_src=False)

        # ================= Phase 4: MLP =================
        with contextlib.ExitStack() as s4:
            wfp = s4.enter_context(tc.tile_pool(name="wfp", bufs=3))
            wprp = s4.enter_context(tc.tile_pool(name="wprp", bufs=2))
            mtp = s4.enter_context(tc.tile_pool(name="mtp", bufs=1))
            frp = s4.enter_context(tc.tile_pool(name="frp", bufs=1))
            osb = s4.enter_context(tc.tile_pool(name="osb", bufs=2))
            pfc = s4.enter_context(tc.tile_pool(name="pfc", bufs=2, space="PSUM"))
            ppr = s4.enter_context(tc.tile_pool(name="ppr", bufs=2, space="PSUM"))

            def fc(c, inline_gelu):
                outs = []
                for hc in range(32):
                    wp = wfp.tile([128, D], bf16, tag="wfpan")
                    nc.sync.dma_start(out=wp[:], in_=wfc_t[hc])
                    ps = pfc.tile([128, 512], f32, tag="pfc")
                    for k in range(KT):
                        nc.tensor.matmul(ps[:], wp[:, k * 128:(k + 1) * 128],
                                         h2_sb[k][:, c * 512:(c + 1) * 512],
                                         start=(k == 0), stop=(k == KT - 1))
                    if inline_gelu:
                        mt = mtp.tile([128, 512], bf16, tag=f"mt{hc}",
                                      name=f"mtB{hc}")
                        nc.scalar.activation(mt[:], ps[:], Act.Gelu,
                                             bias=bfc_t[:, hc:hc + 1])
                    else:
                        fr = frp.tile([128, 512], bf16, tag=f"fr{hc}",
                                      name=f"fr{hc}")
                        nc.vector.tensor_copy(fr[:], ps[:])
                        mt = fr
                    outs.append(mt)
                return outs

            def pr(c, mts, first):
                for mc in range(KT):
                    wp = wprp.tile([128, DFF], bf16, tag="wprpan")
                    nc.sync.dma_start(out=wp[:], in_=wpr_t[mc])
                    ps = ppr.tile([128, 512], f32, tag="ppr")
                    for hc in range(32):
                        nc.tensor.matmul(ps[:], wp[:, hc * 128:(hc + 1) * 128],
                                         mts[hc][:], start=(hc == 0),
                                         stop=(hc == 31))
                    tmp = osb.tile([128, 512], f32, tag="tmp")
                    nc.scalar.activation(tmp[:], ps[:], Act.Identity,
                                         bias=bpr_t[:, mc:mc + 1])
                    o_t = osb.tile([128, 512], f32, tag="osb")
                    nc.vector.tensor_tensor(
                        o_t[:], tmp[:],
                        x1_sb[mc][:, c * 512:(c + 1) * 512],
                        Alu.add)
                    nc.sync.dma_start(
                        out=out_T[mc * 128:(mc + 1) * 128,
                                  c * 512:(c + 1) * 512], in_=o_t[:])

            # chunk A: fc with deferred gelu (keeps exp table resident while
            # the scheduler hoists these matmuls into attention gaps)
            frA = fc(0, inline_gelu=False)
            mtsA = []
            for hc in range(32):
                mt = mtp.tile([128, 512], bf16, tag=f"mt{hc}", name=f"mtA{hc}")
                nc.scalar.activation(mt[:], frA[hc][:], Act.Gelu,
                                     bias=bfc_t[:, hc:hc + 1])
                mtsA.append(mt)
            pr(0, mtsA, first=True)
            mtsB = fc(1, inline_gelu=True)
            pr(1, mtsB, first=False)

        es_h2.close()
        es_x1.close()

    nc.compile()
    _CACHE["nc"] = nc
    return nc


def make_in_maps(x, ln1_g, ln1_b, wq, wk, wv, wo, bo, ln2_g, ln2_b, w_fc, b_fc,
                 w_pr, b_pr):
    import ml_dtypes
    bf = ml_dtypes.bfloat16
    x = np.asarray(x, np.float32)
    g1 = np.asarray(ln1_g, np.float32)
    b1 = np.asarray(ln1_b, np.float32)
    g2 = np.asarray(ln2_g, np.float32)
    b2 = np.asarray(ln2_b, np.float32)
    wq2 = np.transpose(np.asarray(wq, np.float32), (1, 0, 2)).reshape(D, D)
    wk2 = np.transpose(np.asarray(wk, np.float32), (1, 0, 2)).reshape(D, D)
    wv2 = np.transpose(np.asarray(wv, np.float32), (1, 0, 2)).reshape(D, D)
    wo2 = np.asarray(wo, np.float32)
    wfc2 = np.asarray(w_fc, np.float32)
    wpr2 = np.asarray(w_pr, np.float32)

    wq_e = g1[:, None] * wq2
    wk_e = g1[:, None] * wk2
    wv_e = g1[:, None] * wv2
    bq_e = b1 @ wq2
    bk_e = b1 @ wk2
    bv_e = b1 @ wv2
    bo_e = np.asarray(bo, np.float32) + bv_e @ wo2
    wfc_e = g2[:, None] * wfc2
    bfc_e = np.asarray(b_fc, np.float32) + b2 @ wfc2
    bpr_e = np.asarray(b_pr, np.float32)

    def tile_lhs(w):  # [D_in, M] -> [M/128, 128, D_in] panels
        di, m = w.shape
        return np.ascontiguousarray(
            w.reshape(di // 128, 128, m // 128, 128).transpose(2, 1, 0, 3)
            .reshape(m // 128, 128, di), dtype=bf)

    shared = {
        "wq_t": tile_lhs(wq_e), "wk_t": tile_lhs(wk_e),
        "wo_t": tile_lhs(wo2), "wfc_t": tile_lhs(wfc_e),
        "wpr_t": tile_lhs(wpr2),
        "wv_t": np.ascontiguousarray(
            wv_e.reshape(8, 128, 2, 512).transpose(2, 1, 0, 3)
            .reshape(2, 128, 4096), dtype=bf),
        "bq_d": np.ascontiguousarray(bq_e.reshape(8, 128).T),
        "bk_d": np.ascontiguousarray(bk_e.reshape(8, 128).T),
        "bfc_d": np.ascontiguousarray(bfc_e.reshape(32, 128).T),
        "bpr_d": np.ascontiguousarray(bpr_e.reshape(8, 128).T),
        "id128": np.eye(128, dtype=bf),
        "onesc": np.ones((128, 1), bf),
        "onescr": np.ones((128, 1), np.float32),
        "onesr": np.ones((33, 128), bf),
    }
    r = np.arange(128)[:, None]
    c = np.arange(128)[None, :]
    shared["tri"] = np.where(r <= c, 0.0, NEG).astype(bf)

    in_maps = []
    for b in range(4):
        for g in range(2):
            chunks = [0, 3, 1, 2] if g == 0 else [1, 2, 0, 3]
            xb = x[b]  # [T, D]
            xperm = np.concatenate(
                [xb[512 * ch:512 * (ch + 1)] for ch in chunks], 0)
            xTp = np.ascontiguousarray(xperm.T, dtype=bf)  # [D, 2048]
            m = dict(shared)
            m["xT"] = xTp
            m["xres"] = np.ascontiguousarray(
                xperm[:1024].T.astype(np.float32) + bo_e[:, None]).astype(bf)
            m["pb1"] = np.full((128, 1), 0.0 if g == 1 else NEG, np.float32)
            m["pb2"] = np.full((128, 1), 0.0 if g == 0 else NEG, np.float32)
            in_maps.append(m)
    return in_maps


def kernel(x, ln1_g, ln1_b, wq, wk, wv, wo, bo, ln2_g, ln2_b, w_fc, b_fc,
           w_pr, b_pr):
    from concourse.bass_utils import run_bass_kernel_spmd

    nc = _build()
    in_maps = make_in_maps(x, ln1_g, ln1_b, wq, wk, wv, wo, bo, ln2_g, ln2_b,
                           w_fc, b_fc, w_pr, b_pr)
    res = run_bass_kernel_spmd(nc, in_maps, list(range(8)))
    out = np.empty((4, 2048, D), np.float32)
    for b in range(4):
        for g in range(2):
            chunks = [0, 3] if g == 0 else [1, 2]
            o = res.results[2 * b + g]["out_T"]  # [D, 1024]
            for i, ch in enumerate(chunks):
                out[b, 512 * ch:512 * (ch + 1), :] = \
                    o[:, 512 * i:512 * (i + 1)].T
    return out


# revision 3
# speedup vs baseline: 1.0159x; 1.0159x over previous
"""Decoder layer on 8 trn2 cores — v2.

Sharding: core c = 2*b + g. Each core owns batch b and the balanced q-chunk
pair {0,3} (g=0) or {1,2} (g=1) of the 4x512-token chunks, so every core has
identical causal work. Host permutes tokens to local order [A|B|rest1|rest2]
(A = lower own chunk, B = upper own chunk, rest = other core's chunks
ascending); visibility is then uniform:
  chunk A: tiles 0-3 triangular, tiles 8-11 gated by pb1 (data: 0 / -1e30)
  chunk B: tiles 0-3 full, 4-7 triangular, 8-11 full, 12-15 gated by pb2
K/V are computed for all 2048 tokens of the batch (recompute, zero
collectives). Everything runs transposed [D, tokens]; weights arrive as
pre-tiled bf16 panels so each weight DMA is one [128, 1024/4096] transfer.

Packing: scores row-pack head pairs (two K=64 matmuls in one slot), AV
col-packs them (two M=64), softmax denominators ride as packed M=1 matmuls.
V's bias is folded into bo on the host; bo + residual arrive as a
precomputed x_resid input; remaining biases are applied as per-partition
ACT-evac biases. LN rsqrt = exp(-.5*ln(var+eps)) so the whole LN+attention
era uses one ACT table set; the MLP switches once to the gelu set (chunk A's
gelu is deferred past the last exp to avoid table thrash).
"""

import numpy as np

D = 1024
H = 16
DH = 64
TKV = 2048
DFF = 4096
EPS = 1e-5
NEG = -1.0e30
KT = 8  # 1024 / 128

_CACHE = {}

# attention tile lists: (kt, kind) kind: 0=full, 1=triangle, 2=gate1, 3=gate2
VIS_A = [(0, 1), (1, 1), (2, 1), (3, 1), (8, 2), (9, 2), (10, 2), (11, 2)]
VIS_B = ([(k, 0) for k in range(4)] + [(k, 1) for k in range(4, 8)]
         + [(k, 0) for k in range(8, 12)] + [(k, 3) for k in range(12, 16)])


def _build():
    if "nc" in _CACHE:
        return _CACHE["nc"]
    import concourse.mybir as mybir
    import concourse.tile as tile
    from concourse import bacc
    import contextlib

    f32 = mybir.dt.float32
    f32r = mybir.dt.float32r
    bf16 = mybir.dt.bfloat16
    Act = mybir.ActivationFunctionType
    Alu = mybir.AluOpType

    nc = bacc.Bacc(None, target_bir_lowering=False)

    xT = nc.declare_dram_parameter("xT", [D, TKV], bf16, isOutput=False)
    xres = nc.declare_dram_parameter("xres", [D, 1024], bf16, isOutput=False)
    pb1 = nc.declare_dram_parameter("pb1", [128, 1], f32, isOutput=False)
    pb2 = nc.declare_dram_parameter("pb2", [128, 1], f32, isOutput=False)
    tri = nc.declare_dram_parameter("tri", [128, 128], bf16, isOutput=False)
    id128 = nc.declare_dram_parameter("id128", [128, 128], bf16, isOutput=False)
    onesc = nc.declare_dram_parameter("onesc", [128, 1], bf16, isOutput=False)
    onescr = nc.declare_dram_parameter("onescr", [128, 1], f32r, isOutput=False)
    onesr = nc.declare_dram_parameter("onesr", [33, 128], bf16, isOutput=False)
    wq_t = nc.declare_dram_parameter("wq_t", [KT, 128, D], bf16, isOutput=False)
    wk_t = nc.declare_dram_parameter("wk_t", [KT, 128, D], bf16, isOutput=False)
    wv_t = nc.declare_dram_parameter("wv_t", [2, 128, 4096], bf16, isOutput=False)
    wo_t = nc.declare_dram_parameter("wo_t", [KT, 128, D], bf16, isOutput=False)
    wfc_t = nc.declare_dram_parameter("wfc_t", [32, 128, D], bf16, isOutput=False)
    wpr_t = nc.declare_dram_parameter("wpr_t", [KT, 128, DFF], bf16, isOutput=False)
    bq_d = nc.declare_dram_parameter("bq_d", [128, KT], f32, isOutput=False)
    bk_d = nc.declare_dram_parameter("bk_d", [128, KT], f32, isOutput=False)
    bfc_d = nc.declare_dram_parameter("bfc_d", [128, 32], f32, isOutput=False)
    bpr_d = nc.declare_dram_parameter("bpr_d", [128, KT], f32, isOutput=False)
    out_T = nc.declare_dram_parameter("out_T", [D, 1024], f32, isOutput=True)

    with tile.TileContext(nc) as tc, contextlib.ExitStack() as root:
        cst = root.enter_context(tc.tile_pool(name="cst", bufs=1))
        # ---------------- constants
        # NOTE: ones_c/ones_r DMA'd FIRST — with them placed later in the
        # pool, the M=1 bf16 stats matmuls produced corrupted even columns
        # (HW quirk, empirically layout-dependent; see session notes).
        ones_c = cst.tile([128, 1], bf16, tag="onesc")
        nc.sync.dma_start(out=ones_c[:], in_=onesc[:])
        ones_r = cst.tile([33, 128], bf16, tag="onesr")
        nc.sync.dma_start(out=ones_r[:], in_=onesr[:])
        tri_t = cst.tile([128, 128], bf16, tag="tri")
        nc.sync.dma_start(out=tri_t[:], in_=tri[:])
        id_t = cst.tile([128, 128], bf16, tag="id")
        nc.sync.dma_start(out=id_t[:], in_=id128[:])
        ones_cr = cst.tile([128, 1], f32r, tag="onescr")
        nc.sync.dma_start(out=ones_cr[:], in_=onescr[:])
        pb1_t = cst.tile([128, 1], f32, tag="pb1")
        nc.sync.dma_start(out=pb1_t[:], in_=pb1[:])
        pb2_t = cst.tile([128, 1], f32, tag="pb2")
        nc.sync.dma_start(out=pb2_t[:], in_=pb2[:])
        bq_t = cst.tile([128, KT], f32, tag="bq")
        nc.sync.dma_start(out=bq_t[:], in_=bq_d[:])
        bk_t = cst.tile([128, KT], f32, tag="bk")
        nc.sync.dma_start(out=bk_t[:], in_=bk_d[:])
        bfc_t = cst.tile([128, 32], f32, tag="bfc")
        nc.sync.dma_start(out=bfc_t[:], in_=bfc_d[:])
        bpr_t = cst.tile([128, KT], f32, tag="bpr")
        nc.sync.dma_start(out=bpr_t[:], in_=bpr_d[:])
        eps_t = cst.tile([33, 1], f32, tag="eps")
        nc.any.memset(eps_t[:], EPS)

        def layernorm(s, src_tiles, ncols, out_tiles, f32_src):
            """src tiles [128, ncols] -> normalized out tiles (bf16)."""
            nch = ncols // 512
            pst = s.enter_context(tc.tile_pool(name="pst", bufs=1, space="PSUM"))
            pbc = s.enter_context(tc.tile_pool(name="pbc", bufs=2, space="PSUM"))
            stp = s.enter_context(tc.tile_pool(name="stp", bufs=2))
            bcp = s.enter_context(tc.tile_pool(name="bcp", bufs=1))
            sqp = s.enter_context(tc.tile_pool(name="sqp", bufs=1))
            lhs_x = ones_cr if f32_src else ones_c
            mu_bc = bcp.tile([128, ncols], bf16, tag="mubc")
            rs_bc = bcp.tile([128, ncols], bf16, tag="rsbc")
            for half in range((nch + 1) // 2):
                cs = [c for c in (2 * half, 2 * half + 1) if c < nch]
                psx = {c: pst.tile([1, 512], f32, tag=f"psx{c % 2}",
                                   name=f"psx{c}") for c in cs}
                psq = {c: pst.tile([1, 512], f32, tag=f"psq{c % 2}",
                                   name=f"psq{c}") for c in cs}
                for k in range(KT):
                    a = src_tiles[k][:, 1024 * half:1024 * half + 512 * len(cs)]
                    if f32_src:
                        a = a.bitcast(f32)
                    sq = sqp.tile([128, 512 * len(cs)], bf16, tag="sqh")
                    nc.vector.tensor_tensor(sq[:], a, a, Alu.mult)
                    for j, c in enumerate(cs):
                        nc.tensor.matmul(
                            psx[c][:], lhs_x[:],
                            src_tiles[k][:, c * 512:(c + 1) * 512],
                            start=(k == 0), stop=(k == KT - 1))
                        nc.tensor.matmul(
                            psq[c][:], ones_c[:],
                            sq[:, j * 512:(j + 1) * 512],
                            start=(k == 0), stop=(k == KT - 1))
                # stats math immediately per half (frees psum banks)
                for c in cs:
                    ps_x, ps_q = psx[c], psq[c]
                    mu_sb = stp.tile([1, 512], bf16, tag="mu", name=f"mu{c}")
                    nc.scalar.mul(mu_sb[:], ps_x[:], 1.0 / D)
                    t_sb = stp.tile([1, 512], f32, tag="t")
                    nc.vector.tensor_tensor(t_sb[:], ps_x[:], mu_sb[:],
                                            Alu.mult)
                    v_t = stp.tile([1, 512], f32, tag="v")
                    nc.vector.tensor_tensor(v_t[:], ps_q[:], t_sb[:],
                                            Alu.subtract)
                    lnv = stp.tile([1, 512], f32, tag="lnv")
                    nc.scalar.activation(lnv[:], v_t[:], Act.Ln,
                                         bias=eps_t[0:1, 0:1], scale=1.0 / D)
                    rs_sb = stp.tile([1, 512], bf16, tag="rs", name=f"rs{c}")
                    nc.scalar.activation(rs_sb[:], lnv[:], Act.Exp, bias=0.0,
                                         scale=-0.5)
                    pm = pbc.tile([128, 512], f32, tag="pm")
                    nc.tensor.matmul(pm[:], ones_r[0:1, :], mu_sb[:],
                                     start=True, stop=True)
                    nc.vector.tensor_copy(mu_bc[:, c * 512:(c + 1) * 512],
                                          pm[:])
                    pr_ = pbc.tile([128, 512], f32, tag="pm")
                    nc.tensor.matmul(pr_[:], ones_r[0:1, :], rs_sb[:],
                                     start=True, stop=True)
                    nc.vector.tensor_copy(rs_bc[:, c * 512:(c + 1) * 512],
                                          pr_[:])
            # normalize per chunk as soon as that chunk's mu/rs broadcast
            # lands, so projection chains downstream can start on early
            # chunks while later stats still run
            dtp = s.enter_context(tc.tile_pool(name="dtp", bufs=2))
            for c in range(nch):
                sl = slice(c * 512, (c + 1) * 512)
                for k in range(KT):
                    a = src_tiles[k][:, sl]
                    if f32_src:
                        a = a.bitcast(f32)
                    d_t = dtp.tile([128, 512], bf16, tag="d")
                    nc.vector.tensor_tensor(d_t[:], a, mu_bc[:, sl],
                                            Alu.subtract)
                    nc.vector.tensor_tensor(out_tiles[k][:, sl], d_t[:],
                                            rs_bc[:, sl], Alu.mult)

        # long-lived pools, opened in LIFO-close order:
        # x1/h2 close at root exit, attn after WO, kv after attention
        es_x1 = contextlib.ExitStack()
        x1p = es_x1.enter_context(tc.tile_pool(name="x1", bufs=1))
        x1_sb = [x1p.tile([128, 1024], bf16, tag=f"x1{m}", name=f"x1{m}")
                 for m in range(KT)]
        es_h2 = contextlib.ExitStack()
        h2p = es_h2.enter_context(tc.tile_pool(name="h2", bufs=1))
        h2_sb = [h2p.tile([128, 1024], bf16, tag=f"h2{m}", name=f"h2{m}")
                 for m in range(KT)]
        es_att = contextlib.ExitStack()
        attnp = es_att.enter_context(tc.tile_pool(name="attn", bufs=1))
        at_A = [attnp.tile([128, 512], bf16, tag=f"aA{m}", name=f"aA{m}")
                for m in range(KT)]
        at_B = [attnp.tile([128, 512], bf16, tag=f"aB{m}", name=f"aB{m}")
                for m in range(KT)]

        # ================= Phase 1: LN1 + Q/K/V =================
        es_kv = contextlib.ExitStack()
        kvp = es_kv.enter_context(tc.tile_pool(name="kv", bufs=1))
        kT_sb = [kvp.tile([128, TKV], bf16, tag=f"kT{m}", name=f"kT{m}")
                 for m in range(KT)]
        qT_sb = [kvp.tile([128, 1024], bf16, tag=f"qT{m}", name=f"qT{m}")
                 for m in range(KT)]
        v_sb = [kvp.tile([128, D], bf16, tag=f"v{t}", name=f"vt{t}")
                for t in range(16)]

        with contextlib.ExitStack() as s1:
            xp = s1.enter_context(tc.tile_pool(name="xp", bufs=1))
            x_t = []
            for k in range(KT):
                t = xp.tile([128, TKV], bf16, tag=f"x{k}", name=f"x{k}")
                nc.sync.dma_start(out=t[:], in_=xT[k * 128:(k + 1) * 128, :])
                x_t.append(t)
            with contextlib.ExitStack() as s1a:
                # normalize in place: h == x tiles after this
                layernorm(s1a, x_t, TKV, x_t, f32_src=False)
            h_t = x_t

            wpp = s1.enter_context(tc.tile_pool(name="wpp", bufs=3))
            wvp = s1.enter_context(tc.tile_pool(name="wvp", bufs=1))
            pmm = s1.enter_context(tc.tile_pool(name="pmm", bufs=3, space="PSUM"))

            # Q (own 1024 tokens = local cols 0..1023)
            for mc in range(KT):
                wp = wpp.tile([128, D], bf16, tag="wpan")
                nc.sync.dma_start(out=wp[:], in_=wq_t[mc])
                for c in range(2):
                    ps = pmm.tile([128, 512], f32, tag="pmm")
                    for k in range(KT):
                        nc.tensor.matmul(ps[:], wp[:, k * 128:(k + 1) * 128],
                                         h_t[k][:, c * 512:(c + 1) * 512],
                                         start=(k == 0), stop=(k == KT - 1))
                    nc.scalar.activation(qT_sb[mc][:, c * 512:(c + 1) * 512],
                                         ps[:], Act.Identity,
                                         bias=bq_t[:, mc:mc + 1])
            # K (all 2048)
            for mc in range(KT):
                wp = wpp.tile([128, D], bf16, tag="wpan")
                nc.sync.dma_start(out=wp[:], in_=wk_t[mc])
                for c in range(4):
                    ps = pmm.tile([128, 512], f32, tag="pmm")
                    for k in range(KT):
                        nc.tensor.matmul(ps[:], wp[:, k * 128:(k + 1) * 128],
                                         h_t[k][:, c * 512:(c + 1) * 512],
                                         start=(k == 0), stop=(k == KT - 1))
                    nc.scalar.activation(kT_sb[mc][:, c * 512:(c + 1) * 512],
                                         ps[:], Act.Identity,
                                         bias=bk_t[:, mc:mc + 1])
            # V (all 2048 tokens, token-major [tok, D])
            wv_sb = []
            for c in range(2):
                wv_c = wvp.tile([128, 4096], bf16, tag=f"wvpan{c}",
                                name=f"wvpan{c}")
                nc.sync.dma_start(out=wv_c[:], in_=wv_t[c])
                wv_sb.append(wv_c)
            v_order = [0, 1, 2, 3, 8, 9, 10, 11, 4, 5, 6, 7, 12, 13, 14, 15]
            for tl in v_order:
                for c in range(2):
                    ps = pmm.tile([128, 512], f32, tag="pmm")
                    for k in range(KT):
                        nc.tensor.matmul(
                            ps[:], h_t[k][:, tl * 128:(tl + 1) * 128],
                            wv_sb[c][:, k * 512:(k + 1) * 512],
                            start=(k == 0), stop=(k == KT - 1))
                    nc.vector.tensor_copy(v_sb[tl][:, c * 512:(c + 1) * 512],
                                          ps[:])

        # ================= Phase 2: attention =================

        def attn_chunk(pools, vis, qc, at_out):
            # PSUM discipline: one accumulation chain per bank; the first
            # matmul of each chain covers the chain's full column range
            # (mask preloads ACCUMULATE after the scores matmul instead of
            # preceding it).
            pss, pav, pdnb, etp, rcp = pools
            for hp_ in range(KT):
                work = []
                for kt, kind in vis:
                    c0 = (kt % 4) * 128 if kind == 1 else 0
                    ps = pss.tile([128, 1024], f32, tag="pss")
                    for hh in range(2):
                        lo, hi = hh * 64, hh * 64 + 64
                        nc.tensor.matmul(
                            ps[:, 512 * hh + c0:512 * hh + 512],
                            kT_sb[hp_][lo:hi, kt * 128:(kt + 1) * 128],
                            qT_sb[hp_][lo:hi, qc * 512 + c0:qc * 512 + 512],
                            start=True, stop=(kind != 1))
                        if kind == 1:
                            nc.tensor.matmul(
                                ps[:, 512 * hh + c0:512 * hh + c0 + 128],
                                id_t[:], tri_t[:], start=False, stop=True)
                    et = etp.tile([128, 1024], bf16, tag="et")
                    bias = 0.0
                    if kind == 2:
                        bias = pb1_t[:, 0:1]
                    elif kind == 3:
                        bias = pb2_t[:, 0:1]
                    nc.scalar.activation(et[:, c0:1024], ps[:, c0:1024],
                                         Act.Exp, bias=bias, scale=0.125)
                    work.append((kt, c0, et))
                ps_av0 = pav.tile([128, 512], f32, tag="pav0")
                ps_av1 = pav.tile([128, 512], f32, tag="pav1")
                ps_d0 = pdnb.tile([128, 512], f32, tag="dnb", name="psd0")
                ps_d1 = pdnb.tile([128, 512], f32, tag="dnb", name="psd1")
                n = len(work)
                for i, (kt, c0, et) in enumerate(work):
                    st, sp = (i == 0), (i == n - 1)
                    nc.tensor.matmul(
                        ps_av0[0:64, c0:512],
                        v_sb[kt][:, hp_ * 128:hp_ * 128 + 64],
                        et[:, c0:512], start=st, stop=sp)
                    nc.tensor.matmul(
                        ps_av1[64:128, c0:512],
                        v_sb[kt][:, hp_ * 128 + 64:hp_ * 128 + 128],
                        et[:, 512 + c0:1024], start=st, stop=sp)
                    nc.tensor.matmul(ps_d0[0:1, c0:512], ones_c[:],
                                     et[:, c0:512], start=st, stop=sp)
                    nc.tensor.matmul(ps_d1[32:33, c0:512], ones_c[:],
                                     et[:, 512 + c0:1024], start=st, stop=sp)
                rec = rcp.tile([33, 512], bf16, tag="rec")
                with nc.allow_low_precision(reason="softmax denom bf16"):
                    nc.vector.reciprocal(rec[0:1, :], ps_d0[0:1, :])
                    nc.vector.reciprocal(rec[32:33, :], ps_d1[32:33, :])
                ps_b = pdnb.tile([128, 512], f32, tag="dnb", name="psb")
                nc.tensor.matmul(ps_b[0:64, :], ones_r[0:1, 0:64],
                                 rec[0:1, :], start=True, stop=True)
                nc.tensor.matmul(ps_b[64:128, :], ones_r[32:33, 0:64],
                                 rec[32:33, :], start=True, stop=True)
                bc_sb = rcp.tile([128, 512], f32, tag="bcsb")
                nc.vector.tensor_copy(bc_sb[:], ps_b[:])
                nc.vector.tensor_tensor(at_out[hp_][0:64, :], ps_av0[0:64, :],
                                        bc_sb[0:64, :], Alu.mult)
                nc.vector.tensor_tensor(at_out[hp_][64:128, :],
                                        ps_av1[64:128, :],
                                        bc_sb[64:128, :], Alu.mult)

        with contextlib.ExitStack() as s2:
            pss = s2.enter_context(tc.tile_pool(name="pss", bufs=2, space="PSUM"))
            pav = s2.enter_context(tc.tile_pool(name="pav", bufs=1, space="PSUM"))
            pdnb = s2.enter_context(tc.tile_pool(name="pdnb", bufs=2, space="PSUM"))
            etp = s2.enter_context(tc.tile_pool(name="etp", bufs=6))
            rcp = s2.enter_context(tc.tile_pool(name="rcp", bufs=3))
            pools = (pss, pav, pdnb, etp, rcp)
            attn_chunk(pools, VIS_A, 0, at_A)
            attn_chunk(pools, VIS_B, 1, at_B)

        es_kv.close()  # free kT/qT/v

        # ================= Phase 3: WO + LN2 =================
        with contextlib.ExitStack() as s3:
            xrp = s3.enter_context(tc.tile_pool(name="xrp", bufs=1))
            wpp3 = s3.enter_context(tc.tile_pool(name="wpp3", bufs=3))
            pwo = s3.enter_context(tc.tile_pool(name="pwo", bufs=2, space="PSUM"))
            xr_t = []
            for m in range(KT):
                t = xrp.tile([128, 1024], bf16, tag=f"xr{m}", name=f"xr{m}")
                nc.sync.dma_start(out=t[:], in_=xres[m * 128:(m + 1) * 128, :])
                xr_t.append(t)
            for mc in range(KT):
                wp = wpp3.tile([128, D], bf16, tag="wpan3")
                nc.sync.dma_start(out=wp[:], in_=wo_t[mc])
                for c, at_ in enumerate((at_A, at_B)):
                    ps = pwo.tile([128, 512], f32, tag="pwo")
                    for k in range(KT):
                        nc.tensor.matmul(ps[:], wp[:, k * 128:(k + 1) * 128],
                                         at_[k][:], start=(k == 0),
                                         stop=(k == KT - 1))
                    nc.vector.tensor_tensor(
                        x1_sb[mc][:, c * 512:(c + 1) * 512], ps[:],
                        xr_t[mc][:, c * 512:(c + 1) * 512], Alu.add)

        es_att.close()  # free at_A/at_B

        with contextlib.ExitStack() as s3b:
            layernorm(s3b, x1_sb, 1024, h2_sb, f32_src=False)

        # ================= Phase 4: MLP =================
        with contextlib.ExitStack() as s4:
            wfp = s4.enter_context(tc.tile_pool(name="wfp", bufs=3))
            wprp = s4.enter_context(tc.tile_pool(name="wprp", bufs=2))
            mtp = s4.enter_context(tc.tile_pool(name="mtp", bufs=1))
            frp = s4.enter_context(tc.tile_pool(name="frp", bufs=1))
            osb = s4.enter_context(tc.tile_pool(name="osb", bufs=2))
            pfc = s4.enter_context(tc.tile_pool(name="pfc", bufs=2, space="PSUM"))
            ppr = s4.enter_context(tc.tile_pool(name="ppr", bufs=2, space="PSUM"))

            def fc(c, inline_gelu):
                outs = []
                for hc in range(32):
                    wp = wfp.tile([128, D], bf16, tag="wfpan")
                    nc.sync.dma_start(out=wp[:], in_=wfc_t[hc])
                    ps = pfc.tile([128, 512], f32, tag="pfc")
                    for k in range(KT):
                        nc.tensor.matmul(ps[:], wp[:, k * 128:(k + 1) * 128],
                                         h2_sb[k][:, c * 512:(c + 1) * 512],
                                         start=(k == 0), stop=(k == KT - 1))
                    if inline_gelu:
                        mt = mtp.tile([128, 512], bf16, tag=f"mtb{hc}",
                                      name=f"mtB{hc}")
                        nc.scalar.activation(mt[:], ps[:], Act.Gelu,
                                             bias=bfc_t[:, hc:hc + 1])
                    else:
                        fr = frp.tile([128, 512], bf16, tag=f"fr{hc}",
                                      name=f"fr{hc}")
                        nc.vector.tensor_copy(fr[:], ps[:])
                        mt = fr
                    outs.append(mt)
                return outs

            def pr(c, mts, first):
                for mc in range(KT):
                    wp = wprp.tile([128, DFF], bf16, tag="wprpan")
                    nc.sync.dma_start(out=wp[:], in_=wpr_t[mc])
                    ps = ppr.tile([128, 512], f32, tag="ppr")
                    for hc in range(32):
                        nc.tensor.matmul(ps[:], wp[:, hc * 128:(hc + 1) * 128],
                                         mts[hc][:], start=(hc == 0),
                                         stop=(hc == 31))
                    tmp = osb.tile([128, 512], f32, tag="tmp")
                    nc.scalar.activation(tmp[:], ps[:], Act.Identity,
                                         bias=bpr_t[:, mc:mc + 1])
                    o_t = osb.tile([128, 512], f32, tag="osb")
                    nc.vector.tensor_tensor(
                        o_t[:], tmp[:],
                        x1_sb[mc][:, c * 512:(c + 1) * 512],
                        Alu.add)
                    nc.sync.dma_start(
                        out=out_T[mc * 128:(mc + 1) * 128,
                                  c * 512:(c + 1) * 512], in_=o_t[:])

            # chunk A: fc with deferred gelu (keeps exp table resident while
            # the scheduler hoists these matmuls into attention gaps);
            # fc(B) emitted before the gelu(A) batch so its matmuls overlap
            # the ACT-side gelu work
            frA = fc(0, inline_gelu=False)
            mtsB = fc(1, inline_gelu=True)
            mtsA = []
            for hc in range(32):
                mt = mtp.tile([128, 512], bf16, tag=f"mta{hc}", name=f"mtA{hc}")
                nc.scalar.activation(mt[:], frA[hc][:], Act.Gelu,
                                     bias=bfc_t[:, hc:hc + 1])
                mtsA.append(mt)
            pr(0, mtsA, first=True)
            pr(1, mtsB, first=False)

        es_h2.close()
        es_x1.close()

    nc.compile()
    _CACHE["nc"] = nc
    return nc


def make_in_maps(x, ln1_g, ln1_b, wq, wk, wv, wo, bo, ln2_g, ln2_b, w_fc, b_fc,
                 w_pr, b_pr):
    import ml_dtypes
    bf = ml_dtypes.bfloat16
    x = np.asarray(x, np.float32)
    g1 = np.asarray(ln1_g, np.float32)
    b1 = np.asarray(ln1_b, np.float32)
    g2 = np.asarray(ln2_g, np.float32)
    b2 = np.asarray(ln2_b, np.float32)
    wq2 = np.transpose(np.asarray(wq, np.float32), (1, 0, 2)).reshape(D, D)
    wk2 = np.transpose(np.asarray(wk, np.float32), (1, 0, 2)).reshape(D, D)
    wv2 = np.transpose(np.asarray(wv, np.float32), (1, 0, 2)).reshape(D, D)
    wo2 = np.asarray(wo, np.float32)
    wfc2 = np.asarray(w_fc, np.float32)
    wpr2 = np.asarray(w_pr, np.float32)

    wq_e = g1[:, None] * wq2
    wk_e = g1[:, None] * wk2
    wv_e = g1[:, None] * wv2
    bq_e = b1 @ wq2
    bk_e = b1 @ wk2
    bv_e = b1 @ wv2
    bo_e = np.asarray(bo, np.float32) + bv_e @ wo2
    wfc_e = g2[:, None] * wfc2
    bfc_e = np.asarray(b_fc, np.float32) + b2 @ wfc2
    bpr_e = np.asarray(b_pr, np.float32)

    def tile_lhs(w):  # [D_in, M] -> [M/128, 128, D_in] panels
        di, m = w.shape
        return np.ascontiguousarray(
            w.reshape(di // 128, 128, m // 128, 128).transpose(2, 1, 0, 3)
            .reshape(m // 128, 128, di), dtype=bf)

    shared = {
        "wq_t": tile_lhs(wq_e), "wk_t": tile_lhs(wk_e),
        "wo_t": tile_lhs(wo2), "wfc_t": tile_lhs(wfc_e),
        "wpr_t": tile_lhs(wpr2),
        "wv_t": np.ascontiguousarray(
            wv_e.reshape(8, 128, 2, 512).transpose(2, 1, 0, 3)
            .reshape(2, 128, 4096), dtype=bf),
        "bq_d": np.ascontiguousarray(bq_e.reshape(8, 128).T),
        "bk_d": np.ascontiguousarray(bk_e.reshape(8, 128).T),
        "bfc_d": np.ascontiguousarray(bfc_e.reshape(32, 128).T),
        "bpr_d": np.ascontiguousarray(bpr_e.reshape(8, 128).T),
        "id128": np.eye(128, dtype=bf),
        "onesc": np.ones((128, 1), bf),
        "onescr": np.ones((128, 1), np.float32),
        "onesr": np.ones((33, 128), bf),
    }
    r = np.arange(128)[:, None]
    c = np.arange(128)[None, :]
    shared["tri"] = np.where(r <= c, 0.0, NEG).astype(bf)

    in_maps = []
    for b in range(4):
        for g in range(2):
            chunks = [0, 3, 1, 2] if g == 0 else [1, 2, 0, 3]
            xb = x[b]  # [T, D]
            xperm = np.concatenate(
                [xb[512 * ch:512 * (ch + 1)] for ch in chunks], 0)
            xTp = np.ascontiguousarray(xperm.T, dtype=bf)  # [D, 2048]
            m = dict(shared)
            m["xT"] = xTp
            m["xres"] = np.ascontiguousarray(
                xperm[:1024].T.astype(np.float32) + bo_e[:, None]).astype(bf)
            m["pb1"] = np.full((128, 1), 0.0 if g == 1 else NEG, np.float32)
            m["pb2"] = np.full((128, 1), 0.0 if g == 0 else NEG, np.float32)
            in_maps.append(m)
    return in_maps


def kernel(x, ln1_g, ln1_b, wq, wk, wv, wo, bo, ln2_g, ln2_b, w_fc, b_fc,
           w_pr, b_pr):
    from concourse.bass_utils import run_bass_kernel_spmd

    nc = _build()
    in_maps = make_in_maps(x, ln1_g, ln1_b, wq, wk, wv, wo, bo, ln2_g, ln2_b,
                           w_fc, b_fc, w_pr, b_pr)
    res = run_bass_kernel_spmd(nc, in_maps, list(range(8)))
    out = np.empty((4, 2048, D), np.float32)
    for b in range(4):
        for g in range(2):
            chunks = [0, 3] if g == 0 else [1, 2]
            o = res.results[2 * b + g]["out_T"]  # [D, 1024]
            for i, ch in enumerate(chunks):
                out[b, 512 * ch:512 * (ch + 1), :] = \
                    o[:, 512 * i:512 * (i + 1)].T
    return out
